# revision 31
# baseline (speedup 1.0000x reference)
"""GNN message-passing kernel (GCNConv + TransformerConv layer) for 8 Trainium2 cores.

Strategy (edges sharded by dst node; N/8 dst nodes owned per core):
  * h0s = (x @ W_gcn) * dinv computed DATA-PARALLEL over node blocks (fp16), then one
    AllGather -> full h0s on every core (gather source for the GCN sweep).
  * GCN aggregation: per 128-node group, dma_gather h0s[src] rows, build a 0/1
    indicator S[edge, seg] on DVE (iota == segid), segment-sum via PE matmul S^T @ G
    accumulated in PSUM.  Self-loops are extra edges.  dinv[dst] applied on PSUM
    copy-out, +bias, LeakyReLU -> h (f32, SBUF resident).
  * q^T,k,v,s projections per group (fp16 matmuls, lhsT = PE-transposed h).  k|v
    packed fp16 -> kv_local; AllGather CHUNKED and interleaved with the
    aggregation loop so the exchange hides behind compute.
  * alpha: transpose-gather k[src] (channel-major K^T), T = K^T.T @ q^T on PE
    (T[e,s] = <k_src_e, q_node_s>), alpha = rowsum(T * S) -- no per-edge q gather.
    mean/std normalization is scale-invariant so the 1/sqrt(d) factor is dropped.
  * global mean/std via tiny AllReduce of (sum, sumsq); sigmoid -> per-edge scale.
  * output: msg = v[src] * scale segment-summed with the same S-matmul trick, + h @ Ws
    skip; each core writes its rows; host concatenates.

All gathers run on 4 SWDGE queues (descriptor generation parallelizes over Q7 core
pairs); each dma_gather call is <= 8*128 indices (descriptor-ring capacity).
"""

from contextlib import ExitStack

import numpy as np

BF16 = np.float16  # fp16: all values here are O(1)-scaled; 10-bit mantissa beats bf16

# -------------------- problem constants (nn_DimEncoder_19894288515585) ------------
FULL_CFG = dict(N=20000, E=320000, F_IN=1024, H=256, D=128, C=8)
SCALE_PARAM = 3.0
LEAKY_SLOPE = 0.01


def _derive(cfg):
    N, C = cfg["N"], cfg["C"]
    d = dict(cfg)
    assert N % C == 0
    d["NPC"] = NPC = N // C
    d["G"] = G = (NPC + 127) // 128
    d["NPCp"] = NPCp = G * 128
    assert NPCp > NPC, "need junk rows in each block for the guaranteed zero row"
    d["ZROW"] = NPCp - 1
    nb = (N + 1 + 511) // 512
    d["NB"] = NB = ((nb + C - 1) // C) * C          # 512-row node blocks, divisible by C
    d["NBC"] = NB // C
    d["NT"] = NB * 4
    d["NPAD"] = NB * 512
    # kv AllGather chunk boundaries in GROUPS, front-loaded: the last chunk is
    # small so the final (exposed) AllGather is latency-floor only.
    if G >= 8:
        gb = [0, (G * 3) // 8, (G * 6) // 8, G - 1, G]
    elif G >= 2:
        gb = [0, G - 1, G]
    else:
        gb = [0, G]
    d["GB"] = tuple(gb)
    d["NCH"] = len(gb) - 1
    d["CHR"] = NPCp // (len(gb) - 1)                 # unused (kept for compat)
    d["HCH"] = HCH = 4 if (NB * 512 // C) % 4 == 0 else 1
    d["HCR"] = (NB * 512 // C) // HCH                # h0s_loc rows per chunk
    assert cfg["F_IN"] % 128 == 0 and cfg["H"] % 128 == 0
    d["KC"] = cfg["F_IN"] // 128
    d["HC"] = cfg["H"] // 128
    return d


# -------------------- host-side preprocessing --------------------------------------

def _wrap_idx(a):
    """int16 [M] (M%16==0) -> dma_gather index layout [128, M//16]."""
    w = a.reshape(-1, 16).T.astype(np.int16)
    return np.tile(w, (8, 1))


def _group_pack(src_sorted, dst_sorted, rp, cfg, L):
    C, G, NPC = cfg["C"], cfg["G"], cfg["NPC"]
    src_o = np.full((C, G, L), -1, np.int64)
    seg_o = np.full((C, G, L), -1, np.int64)
    msk_o = np.zeros((C, G, L), np.float32)
    for c in range(C):
        for g in range(G):
            n0 = c * NPC + min(128 * g, NPC)
            n1 = c * NPC + min(128 * (g + 1), NPC)
            i0, i1 = rp[n0], rp[n1]
            n = i1 - i0
            assert n <= L
            src_o[c, g, :n] = src_sorted[i0:i1]
            seg_o[c, g, :n] = dst_sorted[i0:i1] - n0
            msk_o[c, g, :n] = 1.0
    return src_o, seg_o, msk_o


def prep_host(inputs, cfg):
    N, E, C = cfg["N"], cfg["E"], cfg["C"]
    NPC, G, NPCp, ZROW = cfg["NPC"], cfg["G"], cfg["NPCp"], cfg["ZROW"]
    NB, NBC, NT, NPAD = cfg["NB"], cfg["NBC"], cfg["NT"], cfg["NPAD"]
    NCH, CHR = cfg["NCH"], cfg["CHR"]
    KC, HC, F, H, D = cfg["KC"], cfg["HC"], cfg["F_IN"], cfg["H"], cfg["D"]

    x = np.asarray(inputs["x"], np.float32)
    ei = np.asarray(inputs["edge_index"])
    src, dst = ei[0].astype(np.int64), ei[1].astype(np.int64)

    cnt = np.bincount(dst, minlength=N)
    rp = np.zeros(N + 1, np.int64)
    rp[1:] = np.cumsum(cnt)

    perm = np.argsort(dst, kind="stable")
    ds, ss = dst[perm], src[perm]

    # sweep A edge set: edges + self loops, re-sorted by dst
    dstA = np.concatenate([ds, np.arange(N, dtype=np.int64)])
    srcA = np.concatenate([ss, np.arange(N, dtype=np.int64)])
    pA = np.argsort(dstA, kind="stable")
    dsA, ssA = dstA[pA], srcA[pA]
    rpA = np.zeros(N + 1, np.int64)
    rpA[1:] = np.cumsum(cnt + 1)

    def max_group_edges(rp_arr):
        m = 0
        for c in range(C):
            for g in range(G):
                n0 = c * NPC + min(128 * g, NPC)
                n1 = c * NPC + min(128 * (g + 1), NPC)
                m = max(m, int(rp_arr[n1] - rp_arr[n0]))
        return m

    TGB = max(1, (max_group_edges(rp) + 127) // 128)

    srcB_p, segB_p, mskB_p = _group_pack(ss, ds, rp, cfg, TGB * 128)

    TGA = max(1, (max_group_edges(rpA) + 127) // 128)
    srcA_p, segA_p, _ = _group_pack(ssA, dsA, rpA, cfg, TGA * 128)
    # h0s_ext uses a 2-chunk interleaved layout (the AllGather is split in two and
    # the first half overlaps phase-1 compute): row = (p//HLF)*HLF*C + c*HLF + p%HLF
    RPC = NPAD // C
    HLF = (NBC // 2) * 512
    if HLF > 0:
        ca, pa = srcA_p // RPC, srcA_p % RPC
        half = (pa >= HLF).astype(np.int64)
        h0row = half * (HLF * C) + ca * (np.where(half == 1, RPC - HLF, HLF)) + \
            (pa - half * HLF)
        idxA = np.where(srcA_p < 0, 0, h0row)
    else:
        idxA = np.where(srcA_p < 0, 0, srcA_p)
    TGAj = [TGA]
    # kv_full row for src node s under the CHUNKED AllGather layout: chunk j covers
    # kv_local rows [GB[j]*128, GB[j+1]*128) and lands at [rowoff[j]*C + c*rows_j, ...).
    GB = cfg["GB"]
    rb = np.array(GB) * 128                                       # row boundaries
    pos = srcB_p % NPC
    cidx = np.searchsorted(rb, pos, side="right") - 1
    rows_j = (rb[1:] - rb[:-1])
    rowoff = np.concatenate([[0], np.cumsum(rows_j * C)])
    kvrow = rowoff[cidx] + (srcB_p // NPC) * rows_j[cidx] + (pos - rb[cidx])
    idxKV = np.where(srcB_p < 0, 0, kvrow)   # pad edges have seg=-1 -> S column zero

    # ---- shared arrays
    xp = np.zeros((NPAD, F), np.float32)
    xp[:N] = x
    xt = np.ascontiguousarray(
        xp.reshape(NB, 512, KC, 128).transpose(0, 3, 2, 1)).astype(BF16)  # [NB,128c,KC,512n]

    wg = np.ascontiguousarray(
        np.asarray(inputs["W_gcn"], np.float32).reshape(KC, 128, H).transpose(1, 0, 2)
    ).astype(BF16)

    def w2(name):
        w = np.asarray(inputs[name], np.float32).reshape(HC, 128, D).transpose(1, 0, 2)
        return np.ascontiguousarray(w).astype(BF16)

    n_idx = np.arange(NPAD)
    rplo = np.where(n_idx < N, rp[np.minimum(n_idx, N - 1)], 0).astype(np.float32)
    rphi = np.where(n_idx < N, rp[np.minimum(n_idx, N - 1) + 1], 0).astype(np.float32)
    rplo = rplo.reshape(NT, 128).T.copy()                         # [128, NT]
    rphi = rphi.reshape(NT, 128).T.copy()

    shared = {
        "wg": wg,
        "wq": w2("Wq"), "wk": w2("Wk"), "wv": w2("Wv"), "ws": w2("Ws"),
        "bg": np.asarray(inputs["b_gcn"], np.float32).reshape(1, H).astype(BF16),
        "bq": np.asarray(inputs["bq"], np.float32).reshape(1, D).astype(BF16),
        "bk": np.asarray(inputs["bk"], np.float32).reshape(1, D).astype(BF16),
        "bv": np.asarray(inputs["bv"], np.float32).reshape(1, D).astype(BF16),
        "bs": np.asarray(inputs["bs"], np.float32).reshape(1, D).astype(BF16),
        "iota": np.tile(np.arange(128, dtype=np.float32)[None, :], (128, 1)).astype(BF16),
        "ident": np.eye(128, dtype=np.float32),
        "identh": np.eye(128, dtype=BF16),
        "ones": np.ones((128, 128), np.float32),
        "onesb": np.ones((1, 128), BF16),
        "validq": (np.arange(128) < (NPC - 128 * (G - 1))).astype(np.float32).reshape(128, 1),
    }

    in_maps = []
    for c in range(C):
        m = dict(shared)
        m["xt"] = np.ascontiguousarray(xt[c * NBC:(c + 1) * NBC])
        m["rplo_p"] = rplo[:, c * NBC * 4:(c + 1) * NBC * 4].copy()
        m["rphi_p"] = rphi[:, c * NBC * 4:(c + 1) * NBC * 4].copy()
        loc = c * NPC + np.arange(NPCp)
        in_core = loc < (c + 1) * NPC
        m["rplo_l"] = np.where(in_core, rp[np.minimum(loc, N - 1)], 0).astype(
            np.float32).reshape(G, 128).T.copy()
        m["rphi_l"] = np.where(in_core, rp[np.minimum(loc, N - 1) + 1], 0).astype(
            np.float32).reshape(G, 128).T.copy()
        m["idxa"] = np.concatenate([_wrap_idx(idxA[c, g]) for g in range(G)], 1)
        m["sega"] = segA_p[c].reshape(G, TGA, 128).transpose(2, 0, 1).reshape(
            128, G * TGA).astype(BF16)
        m["idxkv"] = np.concatenate([_wrap_idx(idxKV[c, g]) for g in range(G)], 1)
        segb_l = segB_p[c].reshape(G, TGB, 128).transpose(2, 0, 1).reshape(128, G * TGB)
        m["segb"] = segb_l.astype(BF16)
        m["segb32"] = segb_l.astype(np.float32)
        m["maskb"] = mskB_p[c].reshape(G, TGB, 128).transpose(2, 0, 1).reshape(
            128, G * TGB).astype(np.float32)
        in_maps.append(m)

    return in_maps, dict(TGA=TGA, TGB=TGB, TGAj=tuple(TGAj))


# -------------------- device program ----------------------------------------------

def build_program(cfg, TGA, TGB, TGAj):
    import concourse.bacc as bacc
    import concourse.mybir as mybir
    from concourse.tile import TileContext

    dt = mybir.dt
    AF = mybir.ActivationFunctionType
    OP = mybir.AluOpType

    N, E, C = cfg["N"], cfg["E"], cfg["C"]
    NPC, G, NPCp = cfg["NPC"], cfg["G"], cfg["NPCp"]
    NBC, NPAD = cfg["NBC"], cfg["NPAD"]
    NCH, GB = cfg["NCH"], cfg["GB"]
    HCH, HCR = cfg["HCH"], cfg["HCR"]
    HLF = (NBC // 2) * 512
    _rb = [b * 128 for b in GB]
    _rowoff = [0]
    for j in range(NCH):
        _rowoff.append(_rowoff[-1] + (_rb[j + 1] - _rb[j]) * C)
    KC, HC, H, D = cfg["KC"], cfg["HC"], cfg["H"], cfg["D"]

    nc = bacc.Bacc("TRN2", target_bir_lowering=False, debug=False, num_devices=C,
                   num_swdge_queues=4)

    def din(name, shape, dtype):
        return nc.dram_tensor(name, list(shape), dtype, kind="ExternalInput").ap()

    xt = din("xt", [NBC, 128, KC, 512], dt.float16)
    wg = din("wg", [128, KC, H], dt.float16)
    wq, wk = din("wq", [128, HC, D], dt.float16), din("wk", [128, HC, D], dt.float16)
    wv, ws = din("wv", [128, HC, D], dt.float16), din("ws", [128, HC, D], dt.float16)
    bg = din("bg", [1, H], dt.float16)
    bq, bk = din("bq", [1, D], dt.float16), din("bk", [1, D], dt.float16)
    bv, bs = din("bv", [1, D], dt.float16), din("bs", [1, D], dt.float16)
    iota = din("iota", [128, 128], dt.float16)
    ident = din("ident", [128, 128], dt.float32)
    identh = din("identh", [128, 128], dt.float16)
    ones = din("ones", [128, 128], dt.float32)
    onesb = din("onesb", [1, 128], dt.float16)
    rplo_p = din("rplo_p", [128, NBC * 4], dt.float32)
    rphi_p = din("rphi_p", [128, NBC * 4], dt.float32)
    rplo_l, rphi_l = din("rplo_l", [128, G], dt.float32), din("rphi_l", [128, G], dt.float32)
    validq = din("validq", [128, 1], dt.float32)
    idxa = din("idxa", [128, G * TGA * 8], dt.int16)
    sega = din("sega", [128, G * TGA], dt.float16)
    idxkv = din("idxkv", [128, G * TGB * 8], dt.int16)
    segb = din("segb", [128, G * TGB], dt.float16)
    segb32 = din("segb32", [128, G * TGB], dt.float32)
    maskb = din("maskb", [128, G * TGB], dt.float32)

    out_l = nc.dram_tensor("out", [NPCp, D], dt.float32, kind="ExternalOutput").ap()

    h0s_loc = nc.dram_tensor("h0s_loc", [NBC * 512, H], dt.float16).ap()
    h0s_ext = nc.dram_tensor("h0s_ext", [NPAD, H], dt.float16, addr_space="Shared").ap()
    kv_local = nc.dram_tensor("kv_local", [NPCp, 2 * D], dt.float16).ap()
    kv_full = nc.dram_tensor("kv_full", [C * NPCp, 2 * D], dt.float16,
                             addr_space="Shared").ap()
    cc_in = nc.dram_tensor("cc_in", [1, 2], dt.float32).ap()
    cc_out = nc.dram_tensor("cc_out", [1, 2], dt.float32, addr_space="Shared").ap()

    groups = [list(range(C))]
    GMAX = 8   # max tiles (x128 idxs) per dma_gather call (SWDGE desc-ring capacity)
    _gq = [0]  # round-robin SWDGE queue: desc gen spreads over Q7 core pairs

    def gather_chunked(out3, src_ap, idx_sb, g, TG, elem, elem_step=None,
                       transpose=False):
        for t0 in range(0, TG, GMAX):
            t1 = min(t0 + GMAX, TG)
            o = out3[:, :, t0 * 128:t1 * 128] if transpose else out3[:, t0:t1, :]
            nc.gpsimd.dma_gather(
                out_ap=o, in_ap=src_ap,
                idxs_ap=idx_sb[:, g * TG * 8 + t0 * 8:g * TG * 8 + t1 * 8],
                num_idxs=(t1 - t0) * 128, num_idxs_reg=(t1 - t0) * 128,
                elem_size=elem, elem_step=elem_step, transpose=transpose,
                queue_num=_gq[0])
            _gq[0] = (_gq[0] + 1) % 4

    with TileContext(nc) as tc, ExitStack() as ctx:
        cpool = ctx.enter_context(tc.tile_pool(name="consts", bufs=1))
        _cn = [0]

        def load_const(ap_in, shape, dtype):
            _cn[0] += 1
            t = cpool.tile(shape, dtype, tag=f"const{_cn[0]}")
            nc.sync.dma_start(out=t[:], in_=ap_in)
            return t

        wg_sb = load_const(wg, [128, KC, H], dt.float16)
        w_sb = {n: load_const(a, [128, HC, D], dt.float16)
                for n, a in (("q", wq), ("k", wk), ("v", wv), ("s", ws))}
        b_sb = {n: load_const(a, [1, D], dt.float16)
                for n, a in (("q", bq), ("k", bk), ("v", bv), ("s", bs))}
        bg_sb = load_const(bg, [1, H], dt.float16)
        iota_sb = load_const(iota, [128, 128], dt.float16)
        ident_sb = load_const(ident, [128, 128], dt.float32)
        identh_sb = load_const(identh, [128, 128], dt.float16)
        ones_sb = load_const(ones, [128, 128], dt.float32)
        onesb_sb = load_const(onesb, [1, 128], dt.float16)
        idxa_sb = load_const(idxa, [128, G * TGA * 8], dt.int16)
        sega_sb = load_const(sega, [128, G * TGA], dt.float16)
        idxkv_sb = load_const(idxkv, [128, G * TGB * 8], dt.int16)
        segb_sb = load_const(segb, [128, G * TGB], dt.float16)
        segb32_sb = load_const(segb32, [128, G * TGB], dt.float32)
        maskb_sb = load_const(maskb, [128, G * TGB], dt.float32)
        validq_sb = load_const(validq, [128, 1], dt.float32)

        # ---- dinv: deg = rp_hi - rp_lo + 1 ; dinv = 1/sqrt(deg)
        dpool = ctx.enter_context(tc.tile_pool(name="dinv", bufs=1))
        dinv_p = dpool.tile([128, NBC * 4], dt.float32)
        dinv_l = dpool.tile([128, G], dt.float32)
        for (lo, hi, dst_t, n) in ((rplo_p, rphi_p, dinv_p, NBC * 4),
                                   (rplo_l, rphi_l, dinv_l, G)):
            lo_t = dpool.tile([128, n], dt.float32, tag="rp_lo")
            hi_t = dpool.tile([128, n], dt.float32, tag="rp_hi")
            nc.sync.dma_start(out=lo_t[:], in_=lo)
            nc.sync.dma_start(out=hi_t[:], in_=hi)
            nc.vector.tensor_tensor(out=hi_t[:], in0=hi_t[:], in1=lo_t[:], op=OP.subtract)
            nc.vector.tensor_scalar_add(hi_t[:], hi_t[:], 1.0)
            nc.scalar.activation(hi_t[:], hi_t[:], AF.Sqrt)
            nc.vector.reciprocal(dst_t[:], hi_t[:])

        # ---- b_gcn broadcast to 128 rows
        with tc.tile_pool(name="psb", bufs=1, space="PSUM") as psb:
            pb = psb.tile([128, H], dt.float32)
            nc.tensor.matmul(pb[:], lhsT=onesb_sb[:1, :], rhs=bg_sb[:1, :],
                             start=True, stop=True)
            bgb_sb = cpool.tile([128, H], dt.float32)
            nc.vector.tensor_copy(bgb_sb[:], pb[:])

        # ================= phase 1: h0s node-block shard + AllGather ===============
        with tc.tile_pool(name="xt_p", bufs=3) as xt_p, \
             tc.tile_pool(name="h0ps", bufs=3, space="PSUM") as h0ps, \
             tc.tile_pool(name="h0st", bufs=3) as h0st:
            for tb in range(NBC):
                xtile = xt_p.tile([128, KC, 512], dt.float16)
                nc.sync.dma_start(out=xtile[:], in_=xt[tb])
                hs = h0st.tile([128, 4, H], dt.float16)
                for j in range(4):
                    t = tb * 4 + j
                    ph = h0ps.tile([128, H], dt.float32)
                    for k in range(KC):
                        nc.tensor.matmul(ph[:],
                                         lhsT=xtile[:, k, j * 128:(j + 1) * 128],
                                         rhs=wg_sb[:, k, :],
                                         start=(k == 0), stop=(k == KC - 1))
                    if j % 2 == 0:
                        nc.vector.tensor_scalar(out=hs[:, j, :], in0=ph[:],
                                                scalar1=dinv_p[:, t:t + 1], scalar2=None,
                                                op0=OP.mult)
                    else:
                        nc.scalar.activation(hs[:, j, :], ph[:], AF.Copy,
                                             scale=dinv_p[:, t:t + 1])
                nc.sync.dma_start(
                    out=h0s_loc[tb * 512:(tb + 1) * 512, :].rearrange(
                        "(j p) h -> p j h", p=128),
                    in_=hs[:])
                if HLF > 0 and (tb + 1) * 512 == HLF:
                    nc.gpsimd.collective_compute(
                        "AllGather", mybir.AluOpType.bypass, replica_groups=groups,
                        ins=[h0s_loc[0:HLF, :]], outs=[h0s_ext[0:HLF * C, :]])
        nc.gpsimd.collective_compute(
            "AllGather", mybir.AluOpType.bypass, replica_groups=groups,
            ins=[h0s_loc[HLF:, :]], outs=[h0s_ext[HLF * C:, :]])

        # ============ fused GCN aggregation + layer-2 projections + kv exchange ====
        hpool = ctx.enter_context(tc.tile_pool(name="keep", bufs=1))
        s_all = hpool.tile([128, G, D], dt.float32)
        qT_all = hpool.tile([128, G, 128], dt.float16)

        with tc.tile_pool(name="h_allp", bufs=1) as hap, \
             tc.tile_pool(name="ga", bufs=3) as ga_p, \
             tc.tile_pool(name="sa", bufs=2) as sa_p, \
             tc.tile_pool(name="aps", bufs=2, space="PSUM") as aps, \
             tc.tile_pool(name="ht", bufs=2) as ht_p, \
             tc.tile_pool(name="tps", bufs=2, space="PSUM") as tps, \
             tc.tile_pool(name="qps", bufs=4, space="PSUM") as qps, \
             tc.tile_pool(name="stg", bufs=2) as stg:
            h_all = hap.tile([128, G, H], dt.float32)
            for g in range(G):
                ga = ga_p.tile([128, TGA, H], dt.float16)
                gather_chunked(ga, h0s_ext, idxa_sb, g, TGA, H)
                sg = sa_p.tile([128, TGA, 128], dt.float16)
                nc.vector.tensor_tensor(
                    out=sg[:],
                    in0=iota_sb[:].unsqueeze(1).broadcast_to([128, TGA, 128]),
                    in1=sega_sb[:, g * TGA:(g + 1) * TGA].unsqueeze(2)
                        .broadcast_to([128, TGA, 128]),
                    op=OP.is_equal)
                ph = aps.tile([128, H], dt.float32)
                for t in range(TGA):
                    nc.tensor.matmul(ph[:], lhsT=sg[:, t, :], rhs=ga[:, t, :],
                                     start=(t == 0), stop=(t == TGA - 1))
                # h = LeakyReLU(dinv * agg + b)
                nc.vector.tensor_scalar(out=h_all[:, g, :], in0=ph[:],
                                        scalar1=dinv_l[:, g:g + 1], scalar2=None,
                                        op0=OP.mult)
                nc.vector.tensor_tensor(out=h_all[:, g, :], in0=h_all[:, g, :],
                                        in1=bgb_sb[:], op=OP.add)
                nc.scalar.activation(h_all[:, g, :], h_all[:, g, :], AF.Lrelu,
                                     alpha=LEAKY_SLOPE)
                # ---- layer-2 projections for this group
                ht = ht_p.tile([128, HC, 128], dt.float16)
                for hc in range(HC):
                    pt = tps.tile([128, 128], dt.float32)
                    nc.tensor.transpose(pt[:], h_all[:, g, hc * 128:(hc + 1) * 128],
                                        ident_sb[:])
                    nc.vector.tensor_copy(ht[:, hc, :], pt[:])
                kv_st = stg.tile([128, 2, D], dt.float16, tag="kv_st")
                for name in ("k", "v", "s"):
                    pq = qps.tile([128, D], dt.float32)
                    for hc in range(HC):
                        nc.tensor.matmul(pq[:], lhsT=ht[:, hc, :], rhs=w_sb[name][:, hc, :],
                                         start=(hc == 0), stop=False)
                    nc.tensor.matmul(pq[:], lhsT=onesb_sb[:1, :], rhs=b_sb[name][:1, :],
                                     start=False, stop=True)
                    dst_ap = {"k": kv_st[:, 0, :], "v": kv_st[:, 1, :],
                              "s": s_all[:, g, :]}[name]
                    if g == G - 1 and name != "s":
                        nc.vector.tensor_scalar(out=dst_ap, in0=pq[:],
                                                scalar1=validq_sb[:, 0:1],
                                                scalar2=None, op0=OP.mult)
                    else:
                        nc.vector.tensor_copy(dst_ap, pq[:])
                # q^T (channel-major): qT = Wq^T @ h^T
                pq = qps.tile([128, D], dt.float32)
                for hc in range(HC):
                    nc.tensor.matmul(pq[:], lhsT=w_sb["q"][:, hc, :], rhs=ht[:, hc, :],
                                     start=(hc == 0), stop=False)
                nc.tensor.matmul(pq[:], lhsT=b_sb["q"][:1, :], rhs=onesb_sb[:1, :],
                                 start=False, stop=True)
                nc.scalar.activation(qT_all[:, g, :], pq[:], AF.Copy)
                nc.sync.dma_start(out=kv_local[g * 128:(g + 1) * 128, :],
                                  in_=kv_st[:].rearrange("p a b -> p (a b)"))
                # chunked kv AllGather: fire as soon as a chunk's rows are done
                if (g + 1) in GB[1:]:
                    j = GB[1:].index(g + 1)
                    nc.gpsimd.collective_compute(
                        "AllGather", mybir.AluOpType.bypass, replica_groups=groups,
                        ins=[kv_local[_rb[j]:_rb[j + 1], :]],
                        outs=[kv_full[_rowoff[j]:_rowoff[j + 1], :]])

        # ================= sweep B: alpha ==========================================
        apool = ctx.enter_context(tc.tile_pool(name="alpha", bufs=1))
        alpha_all = apool.tile([128, G * TGB], dt.float32)
        vkeep = apool.tile([128, G * TGB, D], dt.float16)

        with tc.tile_pool(name="ktb", bufs=3) as ktb_p, \
             tc.tile_pool(name="bps", bufs=3, space="PSUM") as bps, \
             tc.tile_pool(name="tsb", bufs=2) as tsb_p, \
             tc.tile_pool(name="sb2", bufs=2) as sb2_p:
            for g in range(G):
                kg = ktb_p.tile([128, TGB, 2 * D], dt.float16)
                gather_chunked(kg, kv_full, idxkv_sb, g, TGB, 2 * D)
                veng = nc.vector if g % 2 == 0 else nc.scalar
                if g % 2 == 0:
                    nc.vector.tensor_copy(vkeep[:, g * TGB:(g + 1) * TGB, :],
                                          kg[:, :, D:2 * D])
                else:
                    nc.scalar.activation(vkeep[:, g * TGB:(g + 1) * TGB, :],
                                         kg[:, :, D:2 * D], AF.Copy)
                sg = sb2_p.tile([128, TGB, 128], dt.float16)
                nc.vector.tensor_tensor(
                    out=sg[:],
                    in0=iota_sb[:].unsqueeze(1).broadcast_to([128, TGB, 128]),
                    in1=segb_sb[:, g * TGB:(g + 1) * TGB].unsqueeze(2)
                        .broadcast_to([128, TGB, 128]),
                    op=OP.is_equal)
                for t in range(TGB):
                    ptk = bps.tile([128, 128], dt.float16, tag="ptk")
                    nc.tensor.transpose(ptk[:], kg[:, t, 0:D], identh_sb[:])
                    kts = tsb_p.tile([128, 128], dt.float16, tag="kts")
                    nc.scalar.activation(kts[:], ptk[:], AF.Copy)
                    pT = bps.tile([128, 128], dt.float32, tag="pT")
                    nc.tensor.matmul(pT[:], lhsT=kts[:], rhs=qT_all[:, g, :],
                                     start=True, stop=True)
                    # alpha[e] = sum_s T[e,s] * S[e,s]  (row-select, fused on DVE)
                    scr = tsb_p.tile([128, 128], dt.float16, tag="scr")
                    nc.vector.scalar_tensor_tensor(
                        out=scr[:], in0=pT[:], scalar=1.0, in1=sg[:, t, :],
                        op0=OP.mult, op1=OP.mult,
                        accum_out=alpha_all[:, g * TGB + t:g * TGB + t + 1])

        # ================= stats + AllReduce + per-edge scale ======================
        with tc.tile_pool(name="st", bufs=1) as st_p, \
             tc.tile_pool(name="stps", bufs=2, space="PSUM") as stps:
            am = st_p.tile([128, G * TGB], dt.float32)
            nc.vector.tensor_tensor(out=am[:], in0=alpha_all[:], in1=maskb_sb[:], op=OP.mult)
            asq = st_p.tile([128, G * TGB], dt.float32)
            nc.vector.tensor_tensor(out=asq[:], in0=am[:], in1=alpha_all[:], op=OP.mult)
            st2 = st_p.tile([128, 2], dt.float32)
            nc.vector.tensor_reduce(out=st2[:, 0:1], in_=am[:], axis=mybir.AxisListType.X,
                                    op=OP.add)
            nc.vector.tensor_reduce(out=st2[:, 1:2], in_=asq[:], axis=mybir.AxisListType.X,
                                    op=OP.add)
            ps1 = stps.tile([1, 2], dt.float32)
            nc.tensor.matmul(ps1[:], lhsT=ones_sb[:, 0:1], rhs=st2[:], start=True, stop=True)
            ccs = st_p.tile([1, 2], dt.float32)
            nc.vector.tensor_copy(ccs[:], ps1[:])
            nc.sync.dma_start(out=cc_in, in_=ccs[:])
            nc.gpsimd.collective_compute(
                "AllReduce", mybir.AluOpType.add, replica_groups=groups,
                ins=[cc_in], outs=[cc_out])
            ccr = st_p.tile([1, 2], dt.float32)
            nc.sync.dma_start(out=ccr[:], in_=cc_out)
            # mu = S1/E ; var = (S2 - S1*mu)/(E-1) ; c = SCALE/sqrt(var)
            mu = st_p.tile([1, 1], dt.float32)
            nc.vector.tensor_scalar(out=mu[:], in0=ccr[:, 0:1], scalar1=1.0 / E,
                                    scalar2=None, op0=OP.mult)
            var = st_p.tile([1, 1], dt.float32)
            nc.vector.tensor_tensor(out=var[:], in0=ccr[:, 0:1], in1=mu[:], op=OP.mult)
            nc.vector.tensor_tensor(out=var[:], in0=ccr[:, 1:2], in1=var[:], op=OP.subtract)
            nc.vector.tensor_scalar(out=var[:], in0=var[:], scalar1=1.0 / (E - 1),
                                    scalar2=None, op0=OP.mult)
            nc.scalar.activation(var[:], var[:], AF.Sqrt)
            cfac = st_p.tile([1, 1], dt.float32)
            nc.vector.reciprocal(cfac[:], var[:])
            nc.vector.tensor_scalar(out=cfac[:], in0=cfac[:], scalar1=float(SCALE_PARAM),
                                    scalar2=None, op0=OP.mult)
            mc = st_p.tile([1, 2], dt.float32)
            nc.vector.tensor_copy(mc[:, 0:1], mu[:])
            nc.vector.tensor_copy(mc[:, 1:2], cfac[:])
            pb2 = stps.tile([128, 2], dt.float32)
            nc.tensor.matmul(pb2[:], lhsT=ones_sb[0:1, :], rhs=mc[:1, :], start=True,
                             stop=True)
            mc_col = st_p.tile([128, 2], dt.float32)
            nc.vector.tensor_copy(mc_col[:], pb2[:])
            # scale = sigmoid((alpha - mu) * c) * mask  (fp16)
            an = st_p.tile([128, G * TGB], dt.float32)
            nc.vector.tensor_scalar(out=an[:], in0=alpha_all[:],
                                    scalar1=mc_col[:, 0:1], scalar2=mc_col[:, 1:2],
                                    op0=OP.subtract, op1=OP.mult)
            nc.scalar.activation(an[:], an[:], AF.Sigmoid)
            nc.vector.tensor_tensor(out=an[:], in0=an[:], in1=maskb_sb[:], op=OP.mult)
            scale_f = apool.tile([128, G * TGB], dt.float32)
            nc.vector.tensor_copy(scale_f[:], an[:])

        # ================= sweep C: output aggregation =============================
        with tc.tile_pool(name="sc", bufs=2) as sc_p, \
             tc.tile_pool(name="vsc", bufs=2) as vsc_p, \
             tc.tile_pool(name="ops", bufs=2, space="PSUM") as ops, \
             tc.tile_pool(name="ot", bufs=2) as ot_p:
            for g in range(G):
                # S''[e,s] = (iota==seg) * sigma_e : indicator and scale fused
                sg = sc_p.tile([128, TGB, 128], dt.float16)
                for t in range(TGB):
                    gt = g * TGB + t
                    nc.vector.tensor_scalar(out=sg[:, t, :], in0=iota_sb[:],
                                            scalar1=segb32_sb[:, gt:gt + 1],
                                            scalar2=scale_f[:, gt:gt + 1],
                                            op0=OP.is_equal, op1=OP.mult)
                po = ops.tile([128, D], dt.float32)
                for t in range(TGB):
                    nc.tensor.matmul(po[:], lhsT=sg[:, t, :],
                                     rhs=vkeep[:, g * TGB + t, :],
                                     start=(t == 0), stop=(t == TGB - 1))
                ot = ot_p.tile([128, D], dt.float32)
                nc.vector.tensor_tensor(out=ot[:], in0=po[:], in1=s_all[:, g, :], op=OP.add)
                nc.sync.dma_start(out=out_l[g * 128:(g + 1) * 128, :], in_=ot[:])

    nc.compile()
    return nc


# -------------------- driver -------------------------------------------------------

_CACHE = {}


def _get_program(cfg, TGA, TGB, TGAj):
    key = (tuple(sorted(cfg.items())), TGA, TGB, TGAj)
    if key not in _CACHE:
        _CACHE[key] = build_program(cfg, TGA, TGB, TGAj)
    return _CACHE[key]


def run(inputs, cfg_base=None, trace=False):
    cfg = _derive(cfg_base or FULL_CFG)
    in_maps, dyn = prep_host(inputs, cfg)
    nc = _get_program(cfg, dyn["TGA"], dyn["TGB"], dyn["TGAj"])
    from concourse.bass_utils import run_bass_kernel_spmd
    res = run_bass_kernel_spmd(nc, in_maps, list(range(cfg["C"])), trace=trace)
    out = np.concatenate(
        [res.results[c]["out"][:cfg["NPC"]] for c in range(cfg["C"])], 0)
    return out.astype(np.float32), res


def kernel(**inputs):
    out, _ = run(inputs)
    return out
